# revision 20
# baseline (speedup 1.0000x reference)
"""Trainium2 Bass kernel for nn_Conv3DNorm (modulated conv3d + demod + lrelu + clamp).

Reference math (styles == ones):
    dcoef[cout] = rsqrt(sum_{cin,kd,kh,kw} weight^2 + 1e-8)
    y = conv3d(x, weight * dcoef, pad=1)            # per-sample, stride 1
    y = leaky_relu(y + bias, 0.2) * sqrt(2)
    y = clip(y, -256, 256)

Sharding: data-parallel over batch. Core i processes sample i (B=8 == n_cores).
Weight/bias replicated. Host prep is layout/dtype only (transpose, pad, cast).

Design notes (from HW traces):
  - conv = 27 accumulated bf16 matmuls per 512-position output chunk
    (chunk = (depth, half-of-H)); 64 chunks; PSUM bank per chunk, 7-bank
    rotation.  bf16 N=512 matmuls run at the roofline (213.3 + ~2.5 ns
    issue = 216 ns); f32r pays +30 ns/MM.  rel err ~2e-3 (gate 2e-2).
  - matmuls whose rhs base is 2-byte-misaligned (odd element) pay +13 ns.
    kw=1 taps never touch the W halo, so they read from `xpad2` (x with
    H-padding only, width 32: 4B-aligned rows); kw=0/2 read the fully
    padded `xpad`.  All bases even -> no penalty anywhere.
  - x is PRE-PADDED ON HOST in both layouts, so every DMA is a contiguous
    line-rate transfer.  On-chip padding was tried twice and lost: DMA
    into a padded layout degenerates to 64B-run descriptors (~20x slow),
    and DVE halo-column memsets are scattered 2B writes costing ~4.4us
    each (read-modify-write), stalling the startup-critical DVE queue.
  - HAM warm-up: the PE runs at 1.2 GHz until ~3.4us of sustained
    activity; throwaway matmuls on memset tiles (ready ~7.5us) bring it
    to 2.4 GHz just as the first real matmul's data lands (~10.5us).
  - dcoef: squares+reduce on DVE (2 ops), one PE matmul with a ones
    vector emitted after chunk 4 (so the in-order PE queue reaches it
    only after the DVE chain finished), then sqrt/reciprocal; the
    epilogue folds dcoef*sqrt2 and bias*sqrt2.
  - epilogue per chunk is 3 DVE ops: u = ps*(sqrt2*dcoef) + sqrt2*bias;
    v = max(0.2*u, u)  (== sqrt2*leaky_relu);  out = clip(v, +-256).
"""

import os
import sys

for _p in (
    "/root/.axon_site",
    "/root/.axon_site/_ro/trn_rl_repo",
    "/root/.axon_site/_ro/pypackages",
):
    if os.path.isdir(_p) and _p not in sys.path:
        sys.path.insert(0, _p)

import numpy as np

import concourse.bass as bass  # noqa: F401
import concourse.mybir as mybir
import concourse.tile as tile
from concourse import bacc
from concourse.bass_utils import run_bass_kernel_spmd

# Problem constants (hardcoded per contract).
B = 8
CIN = 128
COUT = 128
D = H = W = 32
K = 3
NTAPS = K * K * K  # 27
HP = H + 2  # 34
WP = W + 2  # 34
NCHUNK = 64  # output chunks of 512 spatial positions: (d, half-of-H)
EPS = 1e-8
S1 = float(np.sqrt(2.0))  # ACT_GAIN * GAIN
CLAMP = 256.0
ALPHA = 0.2
NWARM = 7  # HAM warm-up matmuls (~3us at the cold 427ns rate)
DC_CHUNK = 4  # emit the dcoef matmul after this chunk

# matmul dtype: "bf16" (roofline) or "f32r" (TF32-like, +30ns/MM)
MM_MODE = os.environ.get("CONV_MM_MODE", "bf16")

LAST_RESULTS = None  # BassKernelResults of the most recent run (for test.py)

_CACHED = {}


def _build_nc(mode: str):
    dt = mybir.dt
    io_dt = {"f32r": dt.float32r, "bf16": dt.bfloat16, "f32": dt.float32}[mode]

    nc = bacc.Bacc("TRN2")
    xp_d = nc.dram_tensor("xp", [CIN, D, HP, WP], io_dt, kind="ExternalInput")
    xp2_d = nc.dram_tensor("xp2", [CIN, D, HP, W], io_dt, kind="ExternalInput")
    w_d = nc.dram_tensor("w", [CIN, NTAPS, COUT], io_dt, kind="ExternalInput")
    b_d = nc.dram_tensor("bias", [COUT, 1], dt.float32, kind="ExternalInput")
    y_d = nc.dram_tensor("y", [COUT, NCHUNK, 512], dt.float32, kind="ExternalOutput")

    def asf32(ap):
        return ap.bitcast(dt.float32) if mode == "f32r" else ap

    with tile.TileContext(nc) as tc:
        with (
            tc.tile_pool(name="big", bufs=1) as big,
            tc.tile_pool(name="small", bufs=1) as small,
            tc.tile_pool(name="sq", bufs=1) as sqp,
            tc.tile_pool(name="epiv", bufs=4) as vp,
            tc.tile_pool(name="epio", bufs=4) as op,
        ):
            xpad = big.tile([CIN, D, HP, WP], io_dt)   # (H,W)-padded
            xpad2 = big.tile([CIN, D, HP, W], io_dt)   # H-padded only
            w_sb = big.tile([CIN, NTAPS, COUT], io_dt)
            bias_sb = small.tile([COUT, 1], dt.float32)

            # warm-up operands (memset, ready ~7.5us -- before any DMA lands)
            warm_w = small.tile([CIN, COUT], io_dt)
            nc.vector.memset(asf32(warm_w[:]), 0.0)
            warm_x = small.tile([CIN, 512], io_dt)
            nc.vector.memset(asf32(warm_x[:]), 0.0)

            # ---- DMAs, ordered for chunk 0 (which runs taps 9..26 first,
            # reading slices d0/d1); both HWDGE rings used, all contiguous.
            def wpiece(g):
                nc.sync.dma_start(
                    w_sb[:, 9 * g : 9 * (g + 1), :], w_d[:, 9 * g : 9 * (g + 1), :]
                )

            def xslice(d):
                nc.sync.dma_start(xpad[:, d], xp_d[:, d])
                nc.scalar.dma_start(xpad2[:, d], xp2_d[:, d])

            wpiece(1)                       # taps 9-17 (chunk 0's first taps)
            nc.scalar.dma_start(bias_sb[:], b_d[:])
            xslice(0)
            xslice(1)
            wpiece(2)                       # taps 18-26
            xslice(2)
            wpiece(0)                       # taps 0-8 (first used by chunk 2)
            for d in range(3, D):
                xslice(d)

            # ---- demodulation coefficients ----
            scal = {}

            def emit_dcoef(warm_ps):
                # acc[cin,cout] = sum_tap w^2 (2 DVE ops), then one matmul
                # with ones reduces over cin: ps_dc[cout,1] = acc.T @ ones.
                ones = small.tile([CIN, 1], dt.float32)
                nc.vector.memset(ones[:], 1.0)
                eps_t = small.tile([COUT, 1], dt.float32)
                nc.vector.memset(eps_t[:], EPS)
                wflat = asf32(w_sb[:]).rearrange("c t f -> c (t f)")
                sq = sqp.tile([CIN, NTAPS * COUT], dt.float32)
                nc.vector.tensor_mul(sq[:], wflat, wflat)
                acc = small.tile([CIN, COUT], dt.float32)
                nc.vector.tensor_reduce(
                    acc[:],
                    sq[:].rearrange("c (t f) -> c f t", t=NTAPS),
                    axis=mybir.AxisListType.X,
                    op=mybir.AluOpType.add,
                )
                ps_dc = warm_ps[:, 0:1]
                nc.tensor.matmul(ps_dc, acc[:], ones[:], start=True, stop=True)
                # dscale = sqrt(2) / sqrt(sums + eps)
                rsq = small.tile([COUT, 1], dt.float32)
                nc.scalar.activation(
                    rsq[:], ps_dc, mybir.ActivationFunctionType.Sqrt, bias=eps_t[:]
                )
                rec = small.tile([COUT, 1], dt.float32)
                nc.vector.reciprocal(rec[:], rsq[:])
                dscale = small.tile([COUT, 1], dt.float32)
                nc.scalar.mul(dscale[:], rec[:], S1)
                bias_s = small.tile([COUT, 1], dt.float32)
                nc.scalar.mul(bias_s[:], bias_sb[:], S1)
                scal["dscale"] = dscale
                scal["bias_s"] = bias_s

            # ---- main conv loop (chunk-major: each chunk's 27 matmuls are
            # consecutive; chunk completions stagger and the epilogues
            # overlap the matmul stream; PE stays at HAM K=8/8 throughout) ----
            with (
                tc.tile_pool(name="ps", bufs=7, space="PSUM") as psp,
                tc.tile_pool(name="dcps", bufs=1, space="PSUM") as dcps,
            ):
                warm_ps = dcps.tile([COUT, 512], dt.float32, name="dc")
                for _ in range(NWARM):
                    nc.tensor.matmul(
                        warm_ps[:], warm_w[:], warm_x[:], start=True, stop=True
                    )

                def epilogue(c, ps):
                    u = vp.tile([COUT, 512], dt.float32, name=f"u_{c}", tag="u")
                    nc.vector.tensor_scalar(
                        out=u[:],
                        in0=ps[:],
                        scalar1=scal["dscale"][:],
                        scalar2=scal["bias_s"][:],
                        op0=mybir.AluOpType.mult,
                        op1=mybir.AluOpType.add,
                    )
                    v = vp.tile([COUT, 512], dt.float32, name=f"v_{c}", tag="v")
                    nc.vector.scalar_tensor_tensor(
                        out=v[:],
                        in0=u[:],
                        scalar=ALPHA,
                        in1=u[:],
                        op0=mybir.AluOpType.mult,
                        op1=mybir.AluOpType.max,
                    )
                    oc = op.tile([COUT, 512], dt.float32, name=f"oc_{c}", tag="oc")
                    nc.vector.tensor_scalar(
                        out=oc[:],
                        in0=v[:],
                        scalar1=-CLAMP,
                        scalar2=CLAMP,
                        op0=mybir.AluOpType.max,
                        op1=mybir.AluOpType.min,
                    )
                    nc.sync.dma_start(y_d[:, c, :], oc[:])

                pending = []
                for c in range(NCHUNK):
                    d, h0 = c // 2, (c % 2) * 16
                    ps = psp.tile([COUT, 512], dt.float32, name=f"ps_{c}", tag="ps")
                    valid = [t for t in range(NTAPS) if 0 <= d + t // 9 - 1 < D]
                    for t in valid:
                        kd, kh, kw = t // 9, (t // 3) % 3, t % 3
                        if kw == 1:
                            rhs = xpad2[:, d + kd - 1, h0 + kh : h0 + kh + 16, :]
                        else:
                            rhs = xpad[
                                :, d + kd - 1, h0 + kh : h0 + kh + 16, kw : kw + 32
                            ]
                        nc.tensor.matmul(
                            ps[:],
                            w_sb[:, t, :],
                            rhs,
                            start=(t == valid[0]),
                            stop=(t == valid[-1]),
                        )
                    if c < DC_CHUNK:
                        # deferred: these epilogues need dcoef, emitted after
                        # chunk DC_CHUNK so the in-order PE queue reaches the
                        # dcoef matmul only after the DVE chain finished.
                        pending.append((c, ps))
                        continue
                    if c == DC_CHUNK:
                        emit_dcoef(warm_ps)
                        for pc, pps in pending:
                            epilogue(pc, pps)
                    epilogue(c, ps)
    nc.compile()
    return nc


def _get_nc(mode: str):
    if mode not in _CACHED:
        _CACHED[mode] = _build_nc(mode)
    return _CACHED[mode]


def kernel(x: np.ndarray, weight: np.ndarray, bias: np.ndarray) -> np.ndarray:
    global LAST_RESULTS
    mode = MM_MODE
    if mode == "bf16":
        import ml_dtypes

        io = ml_dtypes.bfloat16
    else:
        io = np.float32

    x = np.asarray(x)
    weight = np.asarray(weight, dtype=np.float32)
    bias = np.asarray(bias, dtype=np.float32)

    # [cout, cin, kd, kh, kw] -> [cin, (kd kh kw), cout]
    w_prep = np.ascontiguousarray(
        weight.transpose(1, 2, 3, 4, 0).reshape(CIN, NTAPS, COUT).astype(io)
    )
    b_prep = np.ascontiguousarray(bias.reshape(COUT, 1))

    xio = x.astype(io)
    in_maps = []
    for i in range(B):
        xp = np.zeros((CIN, D, HP, WP), dtype=io)
        xp[:, :, 1 : H + 1, 1 : W + 1] = xio[i]
        xp2 = np.zeros((CIN, D, HP, W), dtype=io)
        xp2[:, :, 1 : H + 1, :] = xio[i]
        in_maps.append({"xp": xp, "xp2": xp2, "w": w_prep, "bias": b_prep})

    nc = _get_nc(mode)
    trace = bool(int(os.environ.get("CONV_TRACE", "0")))
    res = run_bass_kernel_spmd(
        nc,
        in_maps,
        core_ids=list(range(B)),
        trace=trace,
    )
    LAST_RESULTS = res
    out = np.stack(
        [r["y"].reshape(COUT, D, H, W) for r in res.results], axis=0
    ).astype(np.float32)
    return out


# revision 22
# speedup vs baseline: 1.0350x; 1.0350x over previous
"""Trainium2 Bass kernel for nn_Conv3DNorm (modulated conv3d + demod + lrelu + clamp).

Reference math (styles == ones):
    dcoef[cout] = rsqrt(sum_{cin,kd,kh,kw} weight^2 + 1e-8)
    y = conv3d(x, weight * dcoef, pad=1)            # per-sample, stride 1
    y = leaky_relu(y + bias, 0.2) * sqrt(2)
    y = clip(y, -256, 256)

Sharding: data-parallel over batch. Core i processes sample i (B=8 == n_cores).
Weight/bias replicated. Host prep is layout/dtype only (transpose, pad, cast).

Design notes (from HW traces):
  - conv = 27 accumulated bf16 matmuls per 512-position output chunk
    (chunk = (depth, half-of-H)); 64 chunks; PSUM bank per chunk, 7-bank
    rotation.  bf16 N=512 matmuls run at the roofline (213.3 + ~2.5 ns
    issue = 216 ns); f32r pays +30 ns/MM.  rel err ~2e-3 (gate 2e-2).
  - matmuls whose rhs base is 2-byte-misaligned (odd element) pay +13 ns.
    kw=1 taps never touch the W halo, so they read from `xpad2` (x with
    H-padding only, width 32: 4B-aligned rows); kw=0/2 read the fully
    padded `xpad`.  All bases even -> no penalty anywhere.
  - x is PRE-PADDED ON HOST in both layouts, so every DMA is a contiguous
    line-rate transfer.  On-chip padding was tried twice and lost: DMA
    into a padded layout degenerates to 64B-run descriptors (~20x slow),
    and DVE halo-column memsets are scattered 2B writes costing ~4.4us
    each (read-modify-write), stalling the startup-critical DVE queue.
  - HAM warm-up: the PE runs at 1.2 GHz until ~3.4us of sustained
    activity; throwaway matmuls on memset tiles (ready ~7.5us) bring it
    to 2.4 GHz just as the first real matmul's data lands (~10.5us).
  - dcoef: squares+reduce on DVE (2 ops), one PE matmul with a ones
    vector emitted after chunk 4 (so the in-order PE queue reaches it
    only after the DVE chain finished), then sqrt/reciprocal; the
    epilogue folds dcoef*sqrt2 and bias*sqrt2.
  - epilogue per chunk is 3 DVE ops: u = ps*(sqrt2*dcoef) + sqrt2*bias;
    v = max(0.2*u, u)  (== sqrt2*leaky_relu);  out = clip(v, +-256).
"""

import os
import sys

for _p in (
    "/root/.axon_site",
    "/root/.axon_site/_ro/trn_rl_repo",
    "/root/.axon_site/_ro/pypackages",
):
    if os.path.isdir(_p) and _p not in sys.path:
        sys.path.insert(0, _p)

import numpy as np

import concourse.bass as bass  # noqa: F401
import concourse.mybir as mybir
import concourse.tile as tile
from concourse import bacc
from concourse.bass_utils import run_bass_kernel_spmd

# Problem constants (hardcoded per contract).
B = 8
CIN = 128
COUT = 128
D = H = W = 32
K = 3
NTAPS = K * K * K  # 27
HP = H + 2  # 34
WP = W + 2  # 34
NCHUNK = 64  # output chunks of 512 spatial positions: (d, half-of-H)
EPS = 1e-8
S1 = float(np.sqrt(2.0))  # ACT_GAIN * GAIN
CLAMP = 256.0
ALPHA = 0.2
NWARM = 12  # HAM warm-up matmuls: span past the first real matmul's data
            # arrival (~12.5us, HBM-contention-bound) so the PE ramp never
            # pauses -- a gap resets the 3.4us sustained-activity window.
DC_CHUNK = 4  # emit the dcoef matmul after this chunk

# matmul dtype: "bf16" (roofline) or "f32r" (TF32-like, +30ns/MM)
MM_MODE = os.environ.get("CONV_MM_MODE", "bf16")

LAST_RESULTS = None  # BassKernelResults of the most recent run (for test.py)

_CACHED = {}


def _build_nc(mode: str):
    dt = mybir.dt
    io_dt = {"f32r": dt.float32r, "bf16": dt.bfloat16, "f32": dt.float32}[mode]

    nc = bacc.Bacc("TRN2")
    xp_d = nc.dram_tensor("xp", [CIN, D, HP, WP], io_dt, kind="ExternalInput")
    xp2_d = nc.dram_tensor("xp2", [CIN, D, HP, W], io_dt, kind="ExternalInput")
    w_d = nc.dram_tensor("w", [CIN, NTAPS, COUT], io_dt, kind="ExternalInput")
    b_d = nc.dram_tensor("bias", [COUT, 1], dt.float32, kind="ExternalInput")
    y_d = nc.dram_tensor("y", [COUT, NCHUNK, 512], dt.float32, kind="ExternalOutput")

    def asf32(ap):
        return ap.bitcast(dt.float32) if mode == "f32r" else ap

    with tile.TileContext(nc) as tc:
        with (
            tc.tile_pool(name="big", bufs=1) as big,
            tc.tile_pool(name="small", bufs=1) as small,
            tc.tile_pool(name="sq", bufs=1) as sqp,
            tc.tile_pool(name="epiv", bufs=4) as vp,
            tc.tile_pool(name="epio", bufs=4) as op,
        ):
            # xpad2 allocated FIRST: with it placed after xpad, the kw=1
            # matmuls streaming from it measured +44ns each (SBUF placement
            # effect, reproducible); this order shows none.
            xpad2 = big.tile([CIN, D, HP, W], io_dt)   # H-padded only
            xpad = big.tile([CIN, D, HP, WP], io_dt)   # (H,W)-padded
            w_sb = big.tile([CIN, NTAPS, COUT], io_dt)
            bias_sb = small.tile([COUT, 1], dt.float32)

            # warm-up operands (memset, ready ~7.5us -- before any DMA lands)
            warm_w = small.tile([CIN, COUT], io_dt)
            nc.vector.memset(asf32(warm_w[:]), 0.0)
            warm_x = small.tile([CIN, 512], io_dt)
            nc.vector.memset(asf32(warm_x[:]), 0.0)

            # ---- DMAs, ordered for chunk 0 (which runs taps 9..26 first,
            # reading slices d0/d1); both HWDGE rings used, all contiguous.
            def wpiece(g):
                nc.sync.dma_start(
                    w_sb[:, 9 * g : 9 * (g + 1), :], w_d[:, 9 * g : 9 * (g + 1), :]
                )

            def xslice(d):
                nc.sync.dma_start(xpad[:, d], xp_d[:, d])
                nc.scalar.dma_start(xpad2[:, d], xp2_d[:, d])

            wpiece(1)                       # taps 9-17 (chunk 0's first taps)
            nc.scalar.dma_start(bias_sb[:], b_d[:])
            xslice(0)
            xslice(1)
            wpiece(2)                       # taps 18-26
            xslice(2)
            wpiece(0)                       # taps 0-8 (first used by chunk 2)
            for d in range(3, D):
                xslice(d)

            # ---- demodulation coefficients ----
            scal = {}

            def emit_dcoef(warm_ps):
                # acc[cin,cout] = sum_tap w^2 (2 DVE ops), then one matmul
                # with ones reduces over cin: ps_dc[cout,1] = acc.T @ ones.
                ones = small.tile([CIN, 1], dt.float32)
                nc.vector.memset(ones[:], 1.0)
                eps_t = small.tile([COUT, 1], dt.float32)
                nc.vector.memset(eps_t[:], EPS)
                wflat = asf32(w_sb[:]).rearrange("c t f -> c (t f)")
                sq = sqp.tile([CIN, NTAPS * COUT], dt.float32)
                nc.vector.tensor_mul(sq[:], wflat, wflat)
                acc = small.tile([CIN, COUT], dt.float32)
                nc.vector.tensor_reduce(
                    acc[:],
                    sq[:].rearrange("c (t f) -> c f t", t=NTAPS),
                    axis=mybir.AxisListType.X,
                    op=mybir.AluOpType.add,
                )
                ps_dc = warm_ps[:, 0:1]
                nc.tensor.matmul(ps_dc, acc[:], ones[:], start=True, stop=True)
                # dscale = sqrt(2) / sqrt(sums + eps)
                rsq = small.tile([COUT, 1], dt.float32)
                nc.scalar.activation(
                    rsq[:], ps_dc, mybir.ActivationFunctionType.Sqrt, bias=eps_t[:]
                )
                rec = small.tile([COUT, 1], dt.float32)
                nc.vector.reciprocal(rec[:], rsq[:])
                dscale = small.tile([COUT, 1], dt.float32)
                nc.scalar.mul(dscale[:], rec[:], S1)
                bias_s = small.tile([COUT, 1], dt.float32)
                nc.scalar.mul(bias_s[:], bias_sb[:], S1)
                scal["dscale"] = dscale
                scal["bias_s"] = bias_s

            # ---- main conv loop (chunk-major: each chunk's 27 matmuls are
            # consecutive; chunk completions stagger and the epilogues
            # overlap the matmul stream; PE stays at HAM K=8/8 throughout) ----
            with (
                tc.tile_pool(name="ps", bufs=7, space="PSUM") as psp,
                tc.tile_pool(name="dcps", bufs=1, space="PSUM") as dcps,
            ):
                warm_ps = dcps.tile([COUT, 512], dt.float32, name="dc")
                for _ in range(NWARM):
                    nc.tensor.matmul(
                        warm_ps[:], warm_w[:], warm_x[:], start=True, stop=True
                    )

                def epilogue(c, ps):
                    u = vp.tile([COUT, 512], dt.float32, name=f"u_{c}", tag="u")
                    nc.vector.tensor_scalar(
                        out=u[:],
                        in0=ps[:],
                        scalar1=scal["dscale"][:],
                        scalar2=scal["bias_s"][:],
                        op0=mybir.AluOpType.mult,
                        op1=mybir.AluOpType.add,
                    )
                    v = vp.tile([COUT, 512], dt.float32, name=f"v_{c}", tag="v")
                    nc.vector.scalar_tensor_tensor(
                        out=v[:],
                        in0=u[:],
                        scalar=ALPHA,
                        in1=u[:],
                        op0=mybir.AluOpType.mult,
                        op1=mybir.AluOpType.max,
                    )
                    oc = op.tile([COUT, 512], dt.float32, name=f"oc_{c}", tag="oc")
                    nc.vector.tensor_scalar(
                        out=oc[:],
                        in0=v[:],
                        scalar1=-CLAMP,
                        scalar2=CLAMP,
                        op0=mybir.AluOpType.max,
                        op1=mybir.AluOpType.min,
                    )
                    nc.sync.dma_start(y_d[:, c, :], oc[:])

                pending = []
                for c in range(NCHUNK):
                    d, h0 = c // 2, (c % 2) * 16
                    ps = psp.tile([COUT, 512], dt.float32, name=f"ps_{c}", tag="ps")
                    valid = [t for t in range(NTAPS) if 0 <= d + t // 9 - 1 < D]
                    for t in valid:
                        kd, kh, kw = t // 9, (t // 3) % 3, t % 3
                        if kw == 1:
                            rhs = xpad2[:, d + kd - 1, h0 + kh : h0 + kh + 16, :]
                        else:
                            rhs = xpad[
                                :, d + kd - 1, h0 + kh : h0 + kh + 16, kw : kw + 32
                            ]
                        nc.tensor.matmul(
                            ps[:],
                            w_sb[:, t, :],
                            rhs,
                            start=(t == valid[0]),
                            stop=(t == valid[-1]),
                        )
                    if c < DC_CHUNK:
                        # deferred: these epilogues need dcoef, emitted after
                        # chunk DC_CHUNK so the in-order PE queue reaches the
                        # dcoef matmul only after the DVE chain finished.
                        pending.append((c, ps))
                        continue
                    if c == DC_CHUNK:
                        emit_dcoef(warm_ps)
                        for pc, pps in pending:
                            epilogue(pc, pps)
                    epilogue(c, ps)
    nc.compile()
    return nc


def _get_nc(mode: str):
    if mode not in _CACHED:
        _CACHED[mode] = _build_nc(mode)
    return _CACHED[mode]


def kernel(x: np.ndarray, weight: np.ndarray, bias: np.ndarray) -> np.ndarray:
    global LAST_RESULTS
    mode = MM_MODE
    if mode == "bf16":
        import ml_dtypes

        io = ml_dtypes.bfloat16
    else:
        io = np.float32

    x = np.asarray(x)
    weight = np.asarray(weight, dtype=np.float32)
    bias = np.asarray(bias, dtype=np.float32)

    # [cout, cin, kd, kh, kw] -> [cin, (kd kh kw), cout]
    w_prep = np.ascontiguousarray(
        weight.transpose(1, 2, 3, 4, 0).reshape(CIN, NTAPS, COUT).astype(io)
    )
    b_prep = np.ascontiguousarray(bias.reshape(COUT, 1))

    xio = x.astype(io)
    in_maps = []
    for i in range(B):
        xp = np.zeros((CIN, D, HP, WP), dtype=io)
        xp[:, :, 1 : H + 1, 1 : W + 1] = xio[i]
        xp2 = np.zeros((CIN, D, HP, W), dtype=io)
        xp2[:, :, 1 : H + 1, :] = xio[i]
        in_maps.append({"xp": xp, "xp2": xp2, "w": w_prep, "bias": b_prep})

    nc = _get_nc(mode)
    trace = bool(int(os.environ.get("CONV_TRACE", "0")))
    res = run_bass_kernel_spmd(
        nc,
        in_maps,
        core_ids=list(range(B)),
        trace=trace,
    )
    LAST_RESULTS = res
    out = np.stack(
        [r["y"].reshape(COUT, D, H, W) for r in res.results], axis=0
    ).astype(np.float32)
    return out
